# revision 52
# baseline (speedup 1.0000x reference)
"""MiniMax-Text-01 lightning attention layer on 8 Trainium2 NeuronCores.

Sharding: core c = 4*b + g handles batch b (of 2) and head-group g (of 4,
8 heads each) for the qkv projection + block-recurrent attention (phase 1+2).
For the norm/gate/out-projection (phase 4), core c handles a STRIDED token
set: slice c (128 tokens) of every (seq-quarter s, batch beta) pair — 4x2
groups of 128 = 1024 rows.  AllToAll chunk s carries seq-quarter s and is
ready after attention block 4s+3, so the 4 chunks fire at 25/50/75/100% of
phase 2 and the exchange overlaps the attention instead of trailing it.
Each chunk slot carries only the destination's 128 tokens (no batch
duplication, no masking on the receive side).

Everything heavy runs in bf16 on the PE (1 cyc/row, same rate as f32r, half
the SBUF/DMA).  The hidden transpose (hidT) is done by the DMA
xbar-transpose engine straight out of DRAM bf16 — zero PE/DVE cost.
Attention processes head PAIRS packed on 128 partitions: one [128,128]
matmul updates both heads' KV state (per-head block decay folded into the
DVE accumulate), and one [128x128 @ 128x256] matmul computes both heads'
inter-block term against a block-diagonal KV.  The intra-block decay and
q-decay multiplies are pair-packed single DVE ops.  normw is folded into
w_out on the host; RMSNorm's rstd is applied as a per-token scale on the
final PSUM->SBUF copy.

Host reassembles the [2,4096,2048] output from the per-core strided rows.
"""

import numpy as np
import ml_dtypes

import concourse.mybir as mybir
import concourse.tile as tile
from concourse import bacc
from concourse.bass_utils import run_bass_kernel_spmd

# ---------------------------------------------------------------- constants
BATCH, SEQ, HID = 2, 4096, 2048
H, D, B = 32, 64, 256
NB = SEQ // B                    # 16 blocks
LAYER_IDX, N_LAYERS = 3, 12
EPS = 1e-5
N_CORES = 8
HG = 4                           # head groups (tensor parallel)
HL = H // HG                     # 8 local heads
NP2 = HL // 2                    # 4 head pairs
TQ = SEQ // HG                   # 1024 tokens per core in phase 4
NKT = HID // 128                 # 16 contraction tiles

F32 = mybir.dt.float32
BF16 = mybir.dt.bfloat16
ACT = mybir.ActivationFunctionType
ALU = mybir.AluOpType
NPBF16 = ml_dtypes.bfloat16

_cached_nc = None


def _decays_np():
    hr = np.arange(1, H + 1, dtype=np.float64)
    s = (1.0 / 2.0 ** (8.0 / H)) ** hr
    s = s * (1.0 - LAYER_IDX / (N_LAYERS - 1) + 1e-5)
    r = np.arange(1, B + 1, dtype=np.float64)
    q_dec = np.exp(-s[:, None] * r[None, :])                 # [H,B]
    k_dec = np.exp(-s[:, None] * (B - r)[None, :])           # [H,B]
    diff = r[:, None] - r[None, :]
    diag = np.where(diff[None] >= 0,
                    np.exp(-s[:, None, None] * diff[None]), 0.0)  # [H,B,B]
    blk = np.exp(-s * B)                                     # [H]
    f = lambda a: np.asarray(a, dtype=np.float32)
    return f(q_dec), f(k_dec), f(diag), f(blk)


def _build(repeat=1):
    from contextlib import ExitStack

    nc = bacc.Bacc("TRN2", target_bir_lowering=False, debug=False,
                   num_devices=N_CORES)

    hidb = nc.dram_tensor("hidb", [SEQ, HID], BF16, kind="ExternalInput").ap()
    hidq = nc.dram_tensor("hidq", [8, 128, HID], BF16, kind="ExternalInput").ap()
    wqk = nc.dram_tensor("wqk", [128, NKT, HL * 2 * D], BF16,
                         kind="ExternalInput").ap()
    wv = nc.dram_tensor("wv", [128, NKT, HL * D], BF16,
                        kind="ExternalInput").ap()
    wgt = nc.dram_tensor("wgt", [NKT, 128, NKT, 128], BF16,
                         kind="ExternalInput").ap()
    wot = nc.dram_tensor("wot", [4, 128, NKT, 512], BF16,
                         kind="ExternalInput").ap()
    ddt = nc.dram_tensor("ddt", [128, HL, 2, B], BF16, kind="ExternalInput").ap()
    qdp = nc.dram_tensor("qdp", [128, NP2, B], BF16, kind="ExternalInput").ap()
    kdc = nc.dram_tensor("kdc", [128, HL, 2], F32, kind="ExternalInput").ap()
    bdp = nc.dram_tensor("bdp", [128, NP2], F32, kind="ExternalInput").ap()
    identb = nc.dram_tensor("identb", [128, 128], BF16,
                            kind="ExternalInput").ap()
    y = nc.dram_tensor("y", [TQ, HID], F32, kind="ExternalOutput").ap()

    with tile.TileContext(nc) as tc, ExitStack() as top:
        constp = top.enter_context(tc.tile_pool(name="const", bufs=1))
        dramp = top.enter_context(tc.tile_pool(name="dram", bufs=1, space="DRAM"))

        ident_sb = constp.tile([128, 128], BF16)
        nc.sync.dma_start(out=ident_sb[:], in_=identb[:])
        kd_sb = constp.tile([128, HL, 2], F32)
        nc.sync.dma_start(out=kd_sb[:], in_=kdc[:])
        bdp_sb = constp.tile([128, NP2], F32)
        nc.sync.dma_start(out=bdp_sb[:], in_=bdp[:])
        ones_col = constp.tile([128, 1], BF16)
        nc.vector.memset(ones_col[:], 1.0)
        ones_f1 = constp.tile([1, 1], F32)
        nc.vector.memset(ones_f1[:], 1.0)
        eps_sb = constp.tile([1, 1], F32)
        nc.vector.memset(eps_sb[:], EPS)

        # exchange buffers: chunk s covers seq-quarter s; slot j carries my
        # batch's tokens [s*1024 + j*128, +128) for my 8 heads (512 rows).
        # Chunks are shipped two-at-a-time (one collective per seq HALF) to
        # halve the per-collective fixed overhead; one tile per half so a
        # half's collective read never WAR-blocks the next half's writes.
        attn_loc = [dramp.tile([N_CORES, 2, HL * D, 128], BF16, tag=f"al{h}",
                               name="attn_loc") for h in range(2)]
        attn_q = [dramp.tile([N_CORES, 2, HL * D, 128], BF16, tag=f"aq{h}",
                             name="attn_q") for h in range(2)]
        # phase-4 read view: [b, g, cs, q, pp, t] (slot = b*4+g, row = q*128+pp)
        attn_q6 = [aq[:].rearrange("(b g) cs (q pp) t -> b g cs q pp t",
                                   b=2, q=4) for aq in attn_q]

        for _rep in range(repeat):
          with ExitStack() as rep:
            # persists across the phase boundary (gate overlaps the A2A)
            p4hold = rep.enter_context(tc.tile_pool(name="p4hold", bufs=1))
            hidT_q = p4hold.tile([128, NKT, TQ], BF16)

            # ------------------------------------------------- phase 1+2
            with ExitStack() as ph1:
                wp = ph1.enter_context(tc.tile_pool(name="wp", bufs=1))
                htp = ph1.enter_context(tc.tile_pool(name="htp", bufs=4))
                qkp = ph1.enter_context(tc.tile_pool(name="qkp", bufs=4))
                vsp = ph1.enter_context(tc.tile_pool(name="vsp", bufs=2))
                stg = ph1.enter_context(tc.tile_pool(name="stg", bufs=1))
                ostg = ph1.enter_context(tc.tile_pool(name="ostg", bufs=4))
                pjps = ph1.enter_context(
                    tc.tile_pool(name="pjps", bufs=2, space="PSUM"))
                sps = ph1.enter_context(
                    tc.tile_pool(name="sps", bufs=2, space="PSUM"))
                trps = ph1.enter_context(
                    tc.tile_pool(name="trps", bufs=2, space="PSUM"))
                ops = ph1.enter_context(
                    tc.tile_pool(name="ops", bufs=1, space="PSUM"))
                kvps = ph1.enter_context(
                    tc.tile_pool(name="kvps", bufs=1, space="PSUM"))

                # first block-pair's hidT before the weights: the first PE
                # work (v matmuls) needs hidT + wv only.  hidT transposes
                # stay on the SP queue (they serialize with collectives, so
                # nothing else urgent may sit behind them there).
                def load_hidT(pr):
                    t = htp.tile([128, NKT, 512], BF16, tag="hidT",
                                 name="hidT")
                    nc.sync.dma_start_transpose(
                        t[:], hidb[pr * 512:(pr + 1) * 512, :])
                    return t

                hidT_tiles = {0: load_hidT(0)}
                wv_sb = wp.tile([128, NKT, HL * D], BF16)
                nc.sync.dma_start(out=wv_sb[:], in_=wv[:])
                hidT_tiles[1] = load_hidT(1)
                wqk_sb = wp.tile([128, NKT, HL * 2 * D], BF16)
                nc.sync.dma_start(out=wqk_sb[:], in_=wqk[:])
                hidT_tiles[2] = load_hidT(2)
                ddt_sb = wp.tile([128, HL, 2, B], BF16)
                nc.sync.dma_start(out=ddt_sb[:], in_=ddt[:])
                qdp_sb = wp.tile([128, NP2, B], BF16)
                nc.sync.dma_start(out=qdp_sb[:], in_=qdp[:])
                # kv state, head-pair block-diagonal: kvb[:, hp, :] is
                # [128 x 128] with head 2hp at [0:64, 0:64] and head 2hp+1
                # at [64:128, 64:128]; off-diagonal blocks stay zero.
                kvb = wp.tile([128, NP2, 128], BF16)
                nc.vector.memset(kvb[:], 0.0)

                hidq_flat = hidq.rearrange("g t f -> (g t) f")

                for pr in range(NB // 2):        # block pairs, 512 tokens
                    hidT = hidT_tiles.pop(pr)
                    if pr + 3 < NB // 2:
                        hidT_tiles[pr + 3] = load_hidT(pr + 3)
                    if pr == 1:
                        # hid_q transposes for the gate: emitted before the
                        # first collective so they never wait on one (chunked
                        # so no single blob hogs the DMA engines)
                        for gq in range(4):
                            nc.sync.dma_start_transpose(
                                hidT_q[:, :, gq * 256:(gq + 1) * 256],
                                hidq_flat[gq * 256:(gq + 1) * 256, :])

                    # v_sd = silu(hidT.T @ w_v): [128 tok, 4, 512 hd]
                    v_sd = vsp.tile([128, 4, HL * D], BF16, tag="v_sd")
                    for t4 in range(4):
                        ps_v = pjps.tile([128, HL * D], F32, tag="psq",
                                         name="ps_v")
                        for k in range(NKT):
                            nc.tensor.matmul(
                                ps_v[:],
                                hidT[:, k, t4 * 128:(t4 + 1) * 128],
                                wv_sb[:, k, :],
                                start=(k == 0), stop=(k == NKT - 1))
                        nc.scalar.activation(v_sd[:, t4, :], ps_v[:], ACT.Silu)

                    # q/k projections for all 4 pairs (tiles live through
                    # both ib passes below)
                    qts, kts = [], []
                    for hp in range(NP2):
                        ps_q = pjps.tile([128, 512], F32, tag="psq",
                                         name="ps_q")
                        for k in range(NKT):
                            nc.tensor.matmul(
                                ps_q[:],
                                wqk_sb[:, k, hp * 128:(hp + 1) * 128],
                                hidT[:, k, :],
                                start=(k == 0), stop=(k == NKT - 1))
                        qTt = qkp.tile([128, 512], BF16, tag="qTt",
                                       name="qTt")
                        nc.scalar.activation(qTt[:], ps_q[:], ACT.Silu)
                        qts.append(qTt)
                        ps_k = pjps.tile([128, 512], F32, tag="psq",
                                         name="ps_k")
                        for k in range(NKT):
                            nc.tensor.matmul(
                                ps_k[:],
                                wqk_sb[:, k,
                                       512 + hp * 128:512 + (hp + 1) * 128],
                                hidT[:, k, :],
                                start=(k == 0), stop=(k == NKT - 1))
                        kTt = qkp.tile([128, 512], BF16, tag="kTt",
                                       name="kTt")
                        nc.scalar.activation(kTt[:], ps_k[:], ACT.Silu)
                        kts.append(kTt)

                    # staging handed from pass A to pass B (covers both ib:
                    # pass A of ib=1 overlaps the DVE tail of ib=0)
                    sT = stg.tile([128, NP2, 2, 2, 2, B], BF16, tag="sT")
                    qdT = stg.tile([128, NP2, 2, B], BF16, tag="qdT")
                    ksd = stg.tile([128, NP2, 2, 2, 128], BF16, tag="ksd")

                    # ---- pass A: kT transpose, scores, q*q_dec
                    for ib in range(2):
                        c0 = ib * B
                        for hp in range(NP2):
                            qTt, kTt = qts[hp], kts[hp]
                            ps_kt4 = trps.tile([128, 2, 2, D], BF16,
                                               tag="pkt")
                            for hh in range(2):
                                pb = hh * D
                                h = hp * 2 + hh
                                ps_s2 = sps.tile([128, 2, B], F32,
                                                 tag="ps_s", name="ps_s2")
                                for jc in range(2):
                                    nc.tensor.transpose(
                                        ps_kt4[:, hh, jc, :],
                                        kTt[pb:pb + D,
                                            c0 + jc * 128:c0 + (jc + 1) * 128],
                                        ident_sb[pb:pb + D, pb:pb + D])
                                    nc.scalar.activation(
                                        ksd[:, hp, ib, jc, pb:pb + D],
                                        ps_kt4[:, hh, jc, :], ACT.Copy,
                                        scale=kd_sb[:, h, jc:jc + 1])
                                    nc.tensor.matmul(
                                        ps_s2[:, jc, :],
                                        kTt[pb:pb + D,
                                            c0 + jc * 128:c0 + (jc + 1) * 128],
                                        qTt[pb:pb + D, c0:c0 + B],
                                        start=True, stop=True)
                                # intra-block decay for this head
                                nc.vector.tensor_mul(
                                    sT[:, hp, ib, hh, :, :], ps_s2[:],
                                    ddt_sb[:, h, :, :])
                            # q * q_dec, both heads in one DVE op
                            nc.vector.tensor_mul(
                                qdT[:, hp, ib, :], qTt[:, c0:c0 + B],
                                qdp_sb[:, hp, :])

                    # ---- pass B: o = inter + intra, kv update
                    for ib in range(2):
                        n = pr * 2 + ib
                        s4, d4 = n // 4, n % 4
                        o_all = ostg.tile([128, NP2, B], BF16, tag="o_all")
                        for hp in range(NP2):
                            ps_o = ops.tile([128, B], F32)
                            # inter-block term for both heads at once
                            # (block-diagonal kv pair vs stacked decayed q)
                            nc.tensor.matmul(
                                ps_o[:], kvb[:, hp, :], qdT[:, hp, ib, :],
                                start=True, stop=False,
                                skip_group_check=True)
                            # intra-block per head (head1 -> psum rows @64)
                            for hh in range(2):
                                pb = hh * D
                                h = hp * 2 + hh
                                for jc in range(2):
                                    nc.tensor.matmul(
                                        ps_o[pb:pb + D, :],
                                        v_sd[:, ib * 2 + jc,
                                             h * D:(h + 1) * D],
                                        sT[:, hp, ib, hh, jc, :],
                                        start=False,
                                        stop=(hh == 1 and jc == 1),
                                        skip_group_check=True)
                            nc.scalar.activation(o_all[:, hp, :], ps_o[:],
                                                 ACT.Copy)

                            # kv <- blk*kv + (k*kd)^T @ v  (pair-packed; the
                            # decay runs on DVE, off-diag junk is ignored)
                            ps_kv = kvps.tile([128, 128], F32)
                            for jc in range(2):
                                nc.tensor.matmul(
                                    ps_kv[:],
                                    ksd[:, hp, ib, jc, :],
                                    v_sd[:, ib * 2 + jc,
                                         hp * 128:(hp + 1) * 128],
                                    start=(jc == 0), stop=(jc == 1))
                            for hh in range(2):
                                pb = hh * D
                                nc.vector.scalar_tensor_tensor(
                                    out=kvb[pb:pb + D, hp, pb:pb + D],
                                    in0=kvb[pb:pb + D, hp, pb:pb + D],
                                    scalar=bdp_sb[pb:pb + D, hp:hp + 1],
                                    in1=ps_kv[pb:pb + D, pb:pb + D],
                                    op0=ALU.mult, op1=ALU.add)
                        # one DMA ships this block's 8 slots x 512 rows
                        # (ACT hwdge queue: never blocked by collectives)
                        for half in range(2):
                            nc.scalar.dma_start(
                                out=attn_loc[s4 // 2][2 * d4 + half,
                                                      s4 % 2, :, :]
                                .rearrange("(hp p) t -> p hp t", hp=4),
                                in_=o_all[:, :,
                                          half * 128:(half + 1) * 128])

                    # exchange half h fires as soon as blocks 8h..8h+7 are
                    # shipped (emitted inline so the Pool queue releases it
                    # at the right time)
                    if pr % 4 == 3:
                        h = pr // 4
                        nc.gpsimd.collective_compute(
                            "AllToAll", ALU.bypass,
                            replica_groups=[list(range(N_CORES))],
                            ins=[attn_loc[h][:].opt()],
                            outs=[attn_q[h][:].opt()])

            # ------------------------------------------------- phase 4
            with ExitStack() as ph4:
                wgsp = ph4.enter_context(tc.tile_pool(name="wgsp", bufs=4))
                atp = ph4.enter_context(tc.tile_pool(name="atp", bufs=1))
                sqp = ph4.enter_context(tc.tile_pool(name="sqp", bufs=3))
                miscp = ph4.enter_context(tc.tile_pool(name="miscp", bufs=1))
                wop = ph4.enter_context(tc.tile_pool(name="wop", bufs=2))
                ystg = ph4.enter_context(tc.tile_pool(name="ystg", bufs=2))
                gps = ph4.enter_context(
                    tc.tile_pool(name="gps", bufs=2, space="PSUM"))
                ssps = ph4.enter_context(
                    tc.tile_pool(name="ssps", bufs=1, space="PSUM"))
                bcps = ph4.enter_context(
                    tc.tile_pool(name="bcps", bufs=1, space="PSUM"))
                yps = ph4.enter_context(
                    tc.tile_pool(name="yps", bufs=2, space="PSUM"))

                gt = atp.tile([128, NKT, TQ], BF16, tag="gt")

                # gate = sigmoid(w_gate.T @ hidT_q): A2A-independent, covers
                # the exchange tail.
                for k in range(NKT):
                    wgs = wgsp.tile([128, NKT, 128], BF16, tag="wgs")
                    nc.sync.dma_start(out=wgs[:], in_=wgt[k])
                    for c2 in range(2):
                        ps_g = gps.tile([128, 512], F32)
                        for kk in range(NKT):
                            nc.tensor.matmul(
                                ps_g[:], wgs[:, kk, :],
                                hidT_q[:, kk, c2 * 512:(c2 + 1) * 512],
                                start=(kk == 0), stop=(kk == NKT - 1))
                        nc.scalar.activation(gt[:, k, c2 * 512:(c2 + 1) * 512],
                                             ps_g[:], ACT.Sigmoid)

                # received attention, feature-major.  Chunks 0-2 load on the
                # SP queue behind the gate-weight stages; chunk 3 (which
                # waits for the last A2A) loads on the ACT queue after the
                # sigmoids so it never holds up the gate.
                attnT = atp.tile([128, NKT, TQ], BF16, tag="attnT")

                def load_attnT(s, eng):
                    # one DMA per (chunk, batch, source-group): q is
                    # stride-regular within a group's 4 k-tiles
                    for b2 in range(2):
                        off = s * 256 + b2 * 128
                        for g_ in range(4):
                            eng.dma_start(
                                out=attnT[:, g_ * 4:(g_ + 1) * 4,
                                          off:off + 128],
                                in_=attn_q6[s // 2][b2, g_, s % 2].rearrange(
                                    "q pp t -> pp q t"))

                for s in range(2):
                    load_attnT(s, nc.sync)
                load_attnT(2, nc.scalar)
                load_attnT(3, nc.scalar)

                # RMSNorm stats (ACT squares, ones-matmul partition reduce)
                # interleaved with the gate multiply into attnG (DVE).
                attnG = atp.tile([128, NKT, TQ], BF16, tag="attnG")
                ps_ss0 = ssps.tile([1, 512], F32, tag="ss0", name="ps_ss0")
                ps_ss1 = ssps.tile([1, 512], F32, tag="ss1", name="ps_ss1")
                for k in range(NKT):
                    for c2, ps_ss in ((0, ps_ss0), (1, ps_ss1)):
                        sq = sqp.tile([128, 512], BF16, tag="sq")
                        nc.scalar.activation(
                            sq[:], attnT[:, k, c2 * 512:(c2 + 1) * 512],
                            ACT.Square)
                        nc.tensor.matmul(ps_ss[:], ones_col[:], sq[:],
                                         start=(k == 0), stop=(k == NKT - 1))
                    nc.vector.tensor_mul(attnG[:, k, :], attnT[:, k, :],
                                         gt[:, k, :])
                sdev = miscp.tile([1, TQ], F32)
                for c2, ps_ss in ((0, ps_ss0), (1, ps_ss1)):
                    nc.scalar.activation(
                        sdev[0:1, c2 * 512:(c2 + 1) * 512], ps_ss[:],
                        ACT.Sqrt, bias=eps_sb[0:1, 0:1], scale=1.0 / HID)
                rstd = miscp.tile([1, TQ], F32)
                nc.vector.reciprocal(rstd[:], sdev[:])
                ps_rt = bcps.tile([128, 8], F32)
                for m in range(8):
                    nc.tensor.matmul(
                        ps_rt[:, m:m + 1], rstd[0:1, m * 128:(m + 1) * 128],
                        ones_f1[0:1, 0:1], start=True, stop=True)
                rstd_t = miscp.tile([128, 8], F32)
                nc.vector.tensor_copy(rstd_t[:], ps_rt[:])

                # out projection: y = attnG.T @ w_out', scaled by rstd
                wo_tiles = {}
                for nn in range(2):
                    wo_tiles[nn] = wop.tile([128, NKT, 512], BF16, tag="wo",
                                            name="wo")
                    nc.scalar.dma_start(out=wo_tiles[nn][:], in_=wot[nn])
                for nn in range(4):
                    if nn + 2 < 4:
                        wo_tiles[nn + 2] = wop.tile([128, NKT, 512], BF16,
                                                    tag="wo", name="wo")
                        nc.scalar.dma_start(out=wo_tiles[nn + 2][:],
                                            in_=wot[nn + 2])
                    wo = wo_tiles.pop(nn)
                    for m in range(8):
                        ps_y = yps.tile([128, 512], F32)
                        for k in range(NKT):
                            nc.tensor.matmul(
                                ps_y[:],
                                attnG[:, k, m * 128:(m + 1) * 128],
                                wo[:, k, :],
                                start=(k == 0), stop=(k == NKT - 1))
                        y_sb = ystg.tile([128, 512], F32)
                        nc.scalar.mul(y_sb[:], ps_y[:], rstd_t[:, m:m + 1])
                        nc.sync.dma_start(
                            out=y[m * 128:(m + 1) * 128,
                                  nn * 512:(nn + 1) * 512],
                            in_=y_sb[:])

    nc.compile()
    return nc


def _in_maps(hidden_states, w_qkv, norm_weight, w_gate, w_out):
    q_dec, k_dec, diag, blk = _decays_np()
    w_qkv_r = np.ascontiguousarray(w_qkv).reshape(HID, H, 3, D)
    w_out_n = (np.asarray(norm_weight, np.float32)[:, None]
               * np.asarray(w_out, np.float32))
    ident = np.eye(128, dtype=np.float32)

    def bf(a):
        return np.ascontiguousarray(np.asarray(a, np.float32).astype(NPBF16))

    # shared (head-independent) tensors
    wgt_h = bf(np.asarray(w_gate, np.float32)
               .reshape(NKT, 128, NKT, 128).transpose(2, 1, 0, 3))
    wot_h = bf(w_out_n.reshape(NKT, 128, 4, 512).transpose(2, 1, 0, 3))
    hid_bf = [bf(hidden_states[b]) for b in range(BATCH)]

    maps = []
    for c in range(N_CORES):
        b, g = divmod(c, HG)
        hs = slice(g * HL, (g + 1) * HL)
        wq = np.ascontiguousarray(w_qkv_r[:, hs, 0, :]).reshape(HID, HL * D)
        wk = np.ascontiguousarray(w_qkv_r[:, hs, 1, :]).reshape(HID, HL * D)
        wqk_full = np.concatenate([wq, wk], axis=1)            # [HID, 1024]
        wv_full = np.ascontiguousarray(
            w_qkv_r[:, hs, 2, :]).reshape(HID, HL * D)
        # phase-4 strided token set: slice c of every (quarter, batch)
        hidq_h = np.stack(
            [hidden_states[beta, s * TQ + c * 128:s * TQ + (c + 1) * 128]
             for s in range(4) for beta in range(BATCH)], axis=0)
        bdp_h = np.empty((128, NP2), np.float32)
        qdp_h = np.empty((128, NP2, B), np.float32)
        for hp in range(NP2):
            bdp_h[:D, hp] = blk[hs][2 * hp]
            bdp_h[D:, hp] = blk[hs][2 * hp + 1]
            qdp_h[:D, hp, :] = q_dec[hs][2 * hp][None, :]
            qdp_h[D:, hp, :] = q_dec[hs][2 * hp + 1][None, :]
        maps.append({
            "hidb": hid_bf[b],
            "hidq": bf(hidq_h),
            "wqk": bf(wqk_full.reshape(NKT, 128, HL * 2 * D)
                      .transpose(1, 0, 2)),
            "wv": bf(wv_full.reshape(NKT, 128, HL * D).transpose(1, 0, 2)),
            "wgt": wgt_h,
            "wot": wot_h,
            "ddt": bf(diag[hs].reshape(HL, B, 2, 128).transpose(3, 0, 2, 1)),
            "qdp": bf(qdp_h),
            "kdc": np.ascontiguousarray(
                k_dec[hs].reshape(HL, 2, 128).transpose(2, 0, 1)),
            "bdp": bdp_h,
            "identb": bf(ident),
        })
    return maps


def _unshard(results):
    """results: list of per-core dicts with 'y' [TQ, HID] -> full output."""
    out = np.empty((BATCH, SEQ, HID), dtype=np.float32)
    for c in range(N_CORES):
        yc = np.asarray(results[c]["y"]).reshape(4, BATCH, 128, HID)
        for s in range(4):
            for beta in range(BATCH):
                out[beta, s * TQ + c * 128:s * TQ + (c + 1) * 128] = \
                    yc[s, beta]
    return out


def kernel(hidden_states, w_qkv, norm_weight, w_gate, w_out):
    global _cached_nc
    hidden_states = np.asarray(hidden_states, dtype=np.float32)
    w_qkv = np.asarray(w_qkv, dtype=np.float32)
    norm_weight = np.asarray(norm_weight, dtype=np.float32)
    w_gate = np.asarray(w_gate, dtype=np.float32)
    w_out = np.asarray(w_out, dtype=np.float32)

    if _cached_nc is None:
        _cached_nc = _build()
    nc = _cached_nc

    maps = _in_maps(hidden_states, w_qkv, norm_weight, w_gate, w_out)
    res = run_bass_kernel_spmd(nc, maps, list(range(N_CORES)))
    return _unshard(res.results)
